# revision 27
# baseline (speedup 1.0000x reference)
"""Trainium2 Bass kernel for masked-LSTM sentence classifier (nn_ABSA_Lstm).

Data-parallel over 8 NeuronCores: 128 sentences per core.
Per core:
  - gather x = emb[sent] via indirect DMA (f32)
  - transpose x on PE into xT K-tiles (bf16), with a constant ones-row
    appended so the gate bias rides along in the x matmul
  - scan t = 0..T-1:
      PSUM gates[128,1200] = xT_t.T @ Wx_aug + hT.T @ Wh   (bf16 matmuls)
      sigmoid(i,f,o) + tanh(g) on ACT, state update on DVE (f32)
      h captured into hout at t == len-1 via one fused mul-add
      h transposed on PE for the next step
  - logit = hout @ Wout + bout (f32 matmul)
"""

import sys

for _p in ("/opt/trn_rl_repo", "/root/.axon_site/_ro/trn_rl_repo"):
    if _p not in sys.path:
        sys.path.append(_p)

import numpy as np

from concourse import bass, mybir
import concourse.tile as tile
from concourse.bass import IndirectOffsetOnAxis
from concourse.bass_utils import run_bass_kernel_spmd
from concourse.masks import make_identity

B, T, V, D, H, C = 1024, 80, 50000, 300, 300, 3
G = 4 * H            # 1200 gate columns, order [i | f | o | g]
DA = D + 1           # 301: ones-row for bias
N_CORES = 8
BC = B // N_CORES    # 128 sentences per core
P = 128

F32 = mybir.dt.float32
BF16 = mybir.dt.bfloat16
I32 = mybir.dt.int32

# contraction tiles for D+1 (x side) and H (h side)
KX = [(0, 128), (128, 256), (256, DA)]       # sizes 128,128,45
KH = [(0, 128), (128, 256), (256, H)]        # sizes 128,128,44
# PSUM bank-aligned N chunks of the 1200 gate columns
NB = [(0, 512), (512, 1024), (1024, G)]

# instruction types that lower to CTRL (single sync-wait slot in this walrus)
_CTRL_TYPES = {"InstDrain", "InstNoOp", "InstHalt", "InstMatmult"}


def _sync_wait(sem_id, value):
    import bass_rust
    return bass_rust.SyncWait(
        sync_type="semaphore", id=sem_id, ant_name=f"splitsem_{sem_id}",
        wait_mode="sem-ge-imm", wait_value=value, wait_reg=None,
    )


def _sync_update(sem_id, mode, value):
    import bass_rust
    return bass_rust.SyncUpdate(
        sync_type="semaphore", id=sem_id, ant_name=f"splitsem_{sem_id}",
        update_mode=mode, update_value=value, update_reg=None,
    )


def _split_multi_waits(nc, spare_sem_ids):
    """walrus caps sync waits per instruction at 1 for every struct we hit.

    Engine instructions: spill excess waits onto single-wait NoOps placed
    just before, on the same engine (engine streams are in-order).

    DMA/queue instructions: a preceding engine NoOp may not order the DGE
    ring, so the spill NoOps perform ALL the original waits and the last one
    increments a dedicated semaphore; the DMA's single wait becomes that
    semaphore. Each such semaphore is decremented back to 0 at the kernel
    tail so repeated NEFF executions stay correct."""
    f = nc.m.functions[0]
    spare = list(spare_sem_ids)
    eng_sem = {}     # engine -> sem id (one per issuing engine, in-order stream)
    eng_count = {}   # engine -> number of increments so far
    for blk in f.blocks:
        out = []
        for ins in blk.instructions:
            si = ins.sync_info
            waits = list(si.on_wait) if si and si.on_wait else []
            if len(waits) <= 1:
                out.append(ins)
                continue
            tname = type(ins).__name__
            is_dma = ("DMA" in tname or "TensorLoad" in tname
                      or "TensorSave" in tname)
            if is_dma:
                eng = ins.engine
                if eng not in eng_sem:
                    eng_sem[eng] = spare.pop()
                    eng_count[eng] = 0
                sid = eng_sem[eng]
                eng_count[eng] += 1
                target = eng_count[eng]
                for j, w in enumerate(waits):
                    nop = mybir.InstNoOp(name=f"nop-dsplit-{nc.next_id()}")
                    nop.engine = eng
                    upd = [_sync_update(sid, "sem-inc", 1)] if j == len(waits) - 1 else []
                    nop.sync_info = mybir.SyncInfo(on_wait=[w], on_update=upd)
                    out.append(nop)
                si.on_wait = [_sync_wait(sid, target)]
            else:
                for w in waits[:-1]:
                    nop = mybir.InstNoOp(name=f"nop-split-{nc.next_id()}")
                    nop.engine = ins.engine
                    nop.sync_info = mybir.SyncInfo(on_wait=[w], on_update=[])
                    out.append(nop)
                si.on_wait = waits[-1:]
            out.append(ins)
        blk.instructions = out
    # tail: restore spilled-DMA semaphores to 0 for repeat executions
    if eng_sem:
        last_blk = f.blocks[-1]
        tail = list(last_blk.instructions)
        for eng, sid in eng_sem.items():
            nop = mybir.InstNoOp(name=f"nop-dclear-{nc.next_id()}")
            nop.engine = mybir.EngineType.SP
            nop.sync_info = mybir.SyncInfo(
                on_wait=[], on_update=[_sync_update(sid, "sem-sub-imm", eng_count[eng])]
            )
            tail.append(nop)
        last_blk.instructions = tail
    return sum(eng_count.values())


def build(t_steps=T, split_waits=True):
    nc = bass.Bass()
    spare_sems = [nc.alloc_semaphore(f"splitspare{i}") for i in range(48)]

    sent_e = nc.declare_dram_parameter("sent", [BC, T], I32, isOutput=False)
    lensm1_e = nc.declare_dram_parameter("lensm1", [BC, 1], F32, isOutput=False)
    emb_e = nc.declare_dram_parameter("emb", [V, D], F32, isOutput=False)
    wx_e = nc.declare_dram_parameter("wx", [3 * P, G], F32, isOutput=False)
    wh_e = nc.declare_dram_parameter("wh", [3 * P, G], F32, isOutput=False)
    wout_e = nc.declare_dram_parameter("wout", [H, C], F32, isOutput=False)
    bout_e = nc.declare_dram_parameter("bout", [1, C], F32, isOutput=False)
    arange_e = nc.declare_dram_parameter("arange", [1, T], F32, isOutput=False)
    out_e = nc.declare_dram_parameter("out", [BC, C], F32, isOutput=True)

    with tile.TileContext(nc) as tc:
        with (
            tc.tile_pool(name="const", bufs=1) as const,
            tc.tile_pool(name="wpool", bufs=1) as wpool,
            tc.tile_pool(name="xtp", bufs=1) as xtp,
            tc.tile_pool(name="gather", bufs=2) as gather,
            tc.tile_pool(name="work", bufs=2) as work,
            tc.tile_pool(name="psum", bufs=2, space="PSUM") as psum,
        ):
            # ---- constants / preloads ----
            ident = const.tile([P, P], F32)
            make_identity(nc, ident)
            identb = const.tile([P, P], BF16)
            nc.vector.tensor_copy(out=identb[:], in_=ident[:])

            sent_sb = const.tile([BC, T], I32)
            nc.sync.dma_start(out=sent_sb[:], in_=sent_e[:])

            lensm1 = const.tile([BC, 1], F32)
            nc.sync.dma_start(out=lensm1[:], in_=lensm1_e[:])

            arange_sb = const.tile([BC, T], F32)
            arange_bcast = bass.AP(
                tensor=arange_e, offset=0, ap=[[0, BC], [1, T]]
            )
            nc.gpsimd.dma_start(out=arange_sb[:], in_=arange_bcast)

            # delta[b,t] = (t == lens[b]-1), as f32
            delta = const.tile([BC, T], F32)
            nc.vector.tensor_scalar(
                out=delta[:], in0=arange_sb[:], scalar1=lensm1[:, 0:1],
                scalar2=None, op0=mybir.AluOpType.is_equal,
            )

            bout_sb = const.tile([BC, C], F32)
            bout_bcast = bass.AP(
                tensor=bout_e, offset=0, ap=[[0, BC], [1, C]]
            )
            nc.gpsimd.dma_start(out=bout_sb[:], in_=bout_bcast)

            # weights -> bf16 K-tiles
            wstage = wpool.tile([P, G], F32, name="wstage", bufs=2)
            wx_t = []
            for k in range(3):
                wxk = wpool.tile([P, G], BF16, name=f"wx{k}")
                st = wpool.tile_like(wstage, name=f"wxs{k}", tag="wstage", bufs=2)
                nc.sync.dma_start(out=st[:], in_=wx_e[k * P : (k + 1) * P, :])
                nc.vector.tensor_copy(out=wxk[:], in_=st[:])
                wx_t.append(wxk)
            wh_t = []
            for k in range(3):
                whk = wpool.tile([P, G], BF16, name=f"wh{k}")
                st = wpool.tile_like(wstage, name=f"whs{k}", tag="wstage", bufs=2)
                nc.sync.dma_start(out=st[:], in_=wh_e[k * P : (k + 1) * P, :])
                nc.vector.tensor_copy(out=whk[:], in_=st[:])
                wh_t.append(whk)
            wout_t = []
            for k, (k0, k1) in enumerate(KH):
                kk = k1 - k0
                wok = wpool.tile([P, C], F32, name=f"wout{k}")
                nc.sync.dma_start(out=wok[:kk, :], in_=wout_e[k0:k1, :])
                wout_t.append(wok)

            # ---- x path: indirect gather casts f32->bf16 straight into a
            # ring of padded staging tiles; xbar DMA transposes run in bursts
            # (xbar-mode transitions serialize against other DMA traffic).
            # Padded cols: col 300 = 1.0 (bias ones-row), 301:383 = 0.
            xt_all = xtp.tile([P, 3, t_steps * P], BF16, name="xt_all")
            NP = 20
            xpads = []
            for i in range(NP):
                xp = const.tile([P, 3 * P], BF16, name=f"xpad{i}")
                nc.vector.memset(xp[:, D : 3 * P], 0.0)
                nc.vector.memset(xp[:, D : D + 1], 1.0)
                xpads.append(xp)

            def prep_gather(t):
                nc.gpsimd.indirect_dma_start(
                    out=xpads[t % NP][:, 0:D], out_offset=None, in_=emb_e[:],
                    in_offset=IndirectOffsetOnAxis(ap=sent_sb[:, t : t + 1], axis=0),
                )

            def prep_transpose(t):
                nc.sync.dma_start_transpose(
                    out=xt_all[:, :, t * P : (t + 1) * P], in_=xpads[t % NP][:]
                )

            def emit_x_mms(t, ps, stop):
                for k in range(3):
                    for nb, (n0, n1) in enumerate(NB):
                        nc.tensor.matmul(
                            out=ps[:, n0:n1],
                            lhsT=xt_all[:, k, t * P : (t + 1) * P],
                            rhs=wx_t[k][:, n0:n1],
                            start=(k == 0),
                            stop=(stop and k == 2),
                        )

            LAG = 14     # gather lookahead (timesteps)
            BURST = 10   # transposes per burst
            LAC = 4      # transpose lookahead beyond burst start
            # first LAC gathers + transposes first, so step 0 starts ASAP;
            # then the rest of the gather lookahead
            for u in range(min(LAC, t_steps)):
                prep_gather(u)
            for u in range(min(LAC, t_steps)):
                prep_transpose(u)
            for u in range(LAC, min(LAG, t_steps)):
                prep_gather(u)

            # ---- scan ----
            hout = const.tile([BC, H], F32)
            nc.vector.memset(hout[:], 0.0)

            # gate column order is [g | i | f | o]:
            #   g = [0:H], i = [H:2H], f = [2H:3H], o = [3H:4H]
            # Three separate PSUM tiles -- [g] [i,f] [o] -- so each ACT op
            # only waits for its own tile's matmuls. g and o are
            # single-buffered (their readers run early), if is
            # double-buffered; with the transpose tile that is exactly
            # 8 PSUM banks.
            psg = psum.tile([BC, H], F32, name="psg", tag="psg", bufs=1)
            pso = psum.tile([BC, H], F32, name="pso", tag="pso", bufs=1)

            def mms_g(ps, lhsT_of, w_t, start, stop):
                for k in range(3):
                    nc.tensor.matmul(out=ps[:, 0:H], lhsT=lhsT_of(k),
                                     rhs=w_t[k][:, 0:H],
                                     start=(start and k == 0),
                                     stop=(stop and k == 2))

            def mms_if(ps, lhsT_of, w_t, start, stop):
                for k in range(3):
                    lh = lhsT_of(k)
                    st = start and k == 0
                    sp = stop and k == 2
                    nc.tensor.matmul(out=ps[:, 0:512], lhsT=lh,
                                     rhs=w_t[k][:, H : H + 512], start=st, stop=sp)
                    nc.tensor.matmul(out=ps[:, 512 : 2 * H], lhsT=lh,
                                     rhs=w_t[k][:, H + 512 : 3 * H], start=st, stop=sp)

            def mms_o(ps, lhsT_of, w_t, start, stop):
                for k in range(3):
                    nc.tensor.matmul(out=ps[:, 0:H], lhsT=lhsT_of(k),
                                     rhs=w_t[k][:, 3 * H : G],
                                     start=(start and k == 0),
                                     stop=(stop and k == 2))

            def xt_of(t):
                return lambda k: xt_all[:, k, t * P : (t + 1) * P]

            c_prev = None
            ht_prev = None
            psif_cur = psum.tile([BC, 512 + H], F32, name="psif0", tag="psif")
            mms_g(psg, xt_of(0), wx_t, start=True, stop=True)
            mms_if(psif_cur, xt_of(0), wx_t, start=True, stop=True)
            mms_o(pso, xt_of(0), wx_t, start=True, stop=True)
            for t in range(t_steps):
                if t + LAG < t_steps:
                    prep_gather(t + LAG)
                if t % BURST == 0:
                    for u in range(t + LAC, min(t + LAC + BURST, t_steps)):
                        prep_transpose(u)

                last = t + 1 >= t_steps
                if ht_prev is not None:
                    hh = ht_prev
                    ht_of = lambda k: hh[:, k, :]
                    mms_g(psg, ht_of, wh_t, start=False, stop=True)
                    mms_if(psif_cur, ht_of, wh_t, start=False, stop=True)
                    mms_o(pso, ht_of, wh_t, start=False, stop=True)
                # next step's x-part for the double-buffered if-tile: early
                # PE filler while ACT/DVE work on t
                psif_next = None
                if not last:
                    psif_next = psum.tile([BC, 512 + H], F32, name="psif",
                                          tag="psif")
                    mms_if(psif_next, xt_of(t + 1), wx_t, start=True, stop=False)

                # nonlinearities, in chain order (bf16 outputs: DVE TT ops
                # hit the 2x packed mode on bf16, worth the small ACT write
                # penalty)
                gt = work.tile([BC, H], BF16, name="gt", tag="gt")
                nc.scalar.activation(
                    out=gt[:], in_=psg[:, 0:H],
                    func=mybir.ActivationFunctionType.Tanh,
                )
                # g-tile is single-buffered: its t+1 x-part must be emitted
                # after its reader
                if not last:
                    mms_g(psg, xt_of(t + 1), wx_t, start=True, stop=False)
                # f first: t2 = f*c only needs f
                sig_f = work.tile([BC, H], BF16, name="sig_f", tag="sig_f")
                nc.scalar.activation(
                    out=sig_f[:], in_=psif_cur[:, H : 2 * H],
                    func=mybir.ActivationFunctionType.Sigmoid,
                )
                sig_i = work.tile([BC, H], BF16, name="sig_i", tag="sig_i")
                nc.scalar.activation(
                    out=sig_i[:], in_=psif_cur[:, 0:H],
                    func=mybir.ActivationFunctionType.Sigmoid,
                )
                sig_o = work.tile([BC, H], BF16, name="sig_o", tag="sig_o")
                nc.scalar.activation(
                    out=sig_o[:], in_=pso[:, 0:H],
                    func=mybir.ActivationFunctionType.Sigmoid,
                )
                if not last:
                    mms_o(pso, xt_of(t + 1), wx_t, start=True, stop=False)

                # h_new padded to 384 cols (uniform 128-wide transposes; Wh
                # rows 300:383 are zero so junk htT rows are harmless, but
                # pad cols must be finite)
                h_new = work.tile([BC, 3 * P], BF16, name="h_new", tag="h_new")
                nc.vector.memset(h_new[:, D : 3 * P], 0.0)

                # c = f*c + i*g
                if c_prev is not None:
                    t2_ = work.tile([BC, H], BF16, name="t2_", tag="t2_")
                    nc.vector.tensor_mul(t2_[:], sig_f[:], c_prev[:])
                t1_ = work.tile([BC, H], BF16, name="t1_", tag="t1_")
                nc.vector.tensor_mul(t1_[:], sig_i[:], gt[:])
                if c_prev is None:
                    c_new = t1_
                else:
                    c_new = work.tile([BC, H], BF16, name="c_new", tag="c_new")
                    nc.vector.tensor_add(c_new[:], t1_[:], t2_[:])

                tc_ = work.tile([BC, H], BF16, name="tc_", tag="tc_")
                nc.scalar.activation(
                    out=tc_[:], in_=c_new[:], func=mybir.ActivationFunctionType.Tanh
                )
                nc.vector.tensor_mul(h_new[:, 0:H], sig_o[:], tc_[:])

                # transpose h for next step (PE -> PSUM -> DVE copies)
                last_copy = None
                if not last:
                    ht = work.tile([P, 3, P], BF16, name="ht", tag="ht")
                    for k in range(3):
                        trp = psum.tile([P, P], BF16, name="trph", tag="trp")
                        nc.tensor.transpose(
                            out=trp[:], in_=h_new[:, k * P : (k + 1) * P],
                            identity=identb[:],
                        )
                        last_copy = nc.vector.tensor_copy(out=ht[:, k, :], in_=trp[:])
                    ht_prev = ht

                # hout += delta_t * h  (off the critical chain; keep it from
                # stealing the DVE slot between the transposes and copies)
                cap = nc.vector.scalar_tensor_tensor(
                    out=hout[:], in0=h_new[:, 0:H], scalar=delta[:, t : t + 1],
                    in1=hout[:], op0=mybir.AluOpType.mult, op1=mybir.AluOpType.add,
                )
                if last_copy is not None:
                    from bass_rust import add_dep_helper
                    add_dep_helper(cap.ins, last_copy.ins, sync=False,
                                   reason="capture after ht copies")
                c_prev = c_new
                psif_cur = psif_next

            # ---- output projection (f32) ----
            hot = work.tile([P, 3, P], F32, name="hot")
            for k, (k0, k1) in enumerate(KH):
                kk = k1 - k0
                trp = psum.tile([P, P], F32, name="trpo", tag="trp")
                nc.tensor.transpose(
                    out=trp[:kk, :], in_=hout[:, k0:k1], identity=ident[:]
                )
                nc.vector.tensor_copy(out=hot[:kk, k, :], in_=trp[:kk, :])
            po = psum.tile([P, P], F32, name="po", tag="trp")
            for k, (k0, k1) in enumerate(KH):
                kk = k1 - k0
                nc.tensor.matmul(
                    out=po[:, 0:C],
                    lhsT=hot[:kk, k, :],
                    rhs=wout_t[k][:kk, :],
                    start=(k == 0),
                    stop=(k == 2),
                )
            logit = work.tile([BC, C], F32, name="logit")
            nc.vector.tensor_add(logit[:], po[:, 0:C], bout_sb[:])
            nc.sync.dma_start(out=out_e[:], in_=logit[:])

    if split_waits:
        _split_multi_waits(nc, [s.num for s in spare_sems])
    return nc


_NC_CACHE = {}


def _get_nc(t_steps=T):
    if t_steps not in _NC_CACHE:
        _NC_CACHE[t_steps] = build(t_steps)
    return _NC_CACHE[t_steps]


def make_in_maps(sent, lens, emb, Wx, Wh, b, Wout, bout):
    # permute gate columns [i|f|g|o] -> [g|i|f|o]
    perm = np.concatenate(
        [np.arange(600, 900), np.arange(0, 300), np.arange(300, 600),
         np.arange(900, 1200)]
    )
    wx_aug = np.concatenate(
        [np.asarray(Wx, np.float32)[:, perm],
         np.asarray(b, np.float32)[perm][None, :],
         np.zeros((384 - D - 1, G), np.float32)], axis=0
    )
    wh_p = np.concatenate(
        [np.asarray(Wh, np.float32)[:, perm],
         np.zeros((384 - H, G), np.float32)], axis=0
    )
    emb = np.ascontiguousarray(np.asarray(emb, np.float32))
    wout = np.ascontiguousarray(np.asarray(Wout, np.float32))
    bout2 = np.asarray(bout, np.float32).reshape(1, C)
    arange = np.arange(T, dtype=np.float32).reshape(1, T)

    in_maps = []
    for i in range(N_CORES):
        sl = slice(i * BC, (i + 1) * BC)
        in_maps.append({
            "sent": np.ascontiguousarray(np.asarray(sent, np.int32)[sl]),
            "lensm1": (np.asarray(lens, np.int32)[sl] - 1).reshape(BC, 1).astype(np.float32),
            "emb": emb,
            "wx": np.ascontiguousarray(wx_aug),
            "wh": wh_p,
            "wout": wout,
            "bout": bout2,
            "arange": arange,
        })
    return in_maps


def kernel(sent, lens, emb, Wx, Wh, b, Wout, bout):
    nc = _get_nc(T)
    in_maps = make_in_maps(sent, lens, emb, Wx, Wh, b, Wout, bout)
    res = run_bass_kernel_spmd(nc, in_maps, core_ids=list(range(N_CORES)))
    out = np.concatenate(
        [res.results[i]["out"] for i in range(N_CORES)], axis=0
    )
    return out.astype(np.float32)
